# revision 1
# baseline (speedup 1.0000x reference)
"""Trainium2 Bass kernel for nn_EnetGnn (gnn_message_passing).

Self-contained: accepts FULL inputs, shards across 8 NeuronCores internally
(core c -> sample c//4, query-quarter c%4), returns the FULL output.

Math (per sample, h = x reshaped [HW, C], X = h.T [C, HW]):
  rgb_idx/ir_idx = top-16 smallest-distance neighbors per point (kNN on the
  3-ch rgb / 1-ch ir pixel features).  The reference MLP on gathered pairs
  factors through the gather:
    rf @ Wg = gather_rgb(h @ (W1+W2)) - gather_ir(h @ W2)
  and relu/bias commute with the max over k, so per branch:
    m = relu(b + max_k( Z_a[idx_a] + Z_bneg[idx_b] ))
  The SE gate is per-channel, so h_{t+1} = gate_t * h_t and iteration 2 only
  rescales the projection weights' rows by gate_1.  Final output:
    out = relu((1 + gamma * g1 * g2) * x).
"""

import numpy as np

N, C, H, W = 2, 128, 64, 64
HW = H * W            # 4096
QPC = HW // 4         # queries per core = 1024
K = 16
NCORES = 8
NQT = QPC // 128      # query tiles per core = 8
CHUNK = 256           # max8 chunk size
NCHUNK = HW // CHUNK  # 16

_CACHE = {}


def _build(debug=False, sim=False, ablate=(), gmode="fp32"):
    import concourse.bacc as bacc
    import concourse.mybir as mybir
    from concourse.tile import TileContext

    f32 = mybir.dt.float32
    bf16 = mybir.dt.bfloat16
    i16 = mybir.dt.int16
    u16 = mybir.dt.uint16
    AF = mybir.ActivationFunctionType
    OP = mybir.AluOpType
    AX = mybir.AxisListType

    nc = bacc.Bacc("TRN2", target_bir_lowering=False, debug=False, num_devices=NCORES)

    # ---------------- I/O ----------------
    x_d = nc.dram_tensor("x", [C, HW], f32, kind="ExternalInput")
    xq_d = nc.dram_tensor("xq", [C, QPC], f32, kind="ExternalInput")
    rgbq_d = nc.dram_tensor("rgbq", [4, QPC], f32, kind="ExternalInput")
    rgbk_d = nc.dram_tensor("rgbk", [4, HW], f32, kind="ExternalInput")
    irq_d = nc.dram_tensor("irq", [2, QPC], f32, kind="ExternalInput")
    irk_d = nc.dram_tensor("irk", [2, HW], f32, kind="ExternalInput")
    wz_d = nc.dram_tensor("wz", [C, 4 * C], f32, kind="ExternalInput")
    brgb_d = nc.dram_tensor("brgb", [C, 1], f32, kind="ExternalInput")
    bir_d = nc.dram_tensor("bir", [C, 1], f32, kind="ExternalInput")
    se1w_d = nc.dram_tensor("se1w", [2 * C, C // 16], f32, kind="ExternalInput")
    se1b_d = nc.dram_tensor("se1b", [C // 16, 1], f32, kind="ExternalInput")
    se2w_d = nc.dram_tensor("se2w", [C // 16, C], f32, kind="ExternalInput")
    se2b_d = nc.dram_tensor("se2b", [C, 1], f32, kind="ExternalInput")
    gamma_d = nc.dram_tensor("gammav", [1, 1], f32, kind="ExternalInput")
    out_d = nc.dram_tensor("out", [C, QPC], f32, kind="ExternalOutput")
    if debug:
        dbg_idx = nc.dram_tensor("d_idx", [2, 16, QPC], f32, kind="ExternalOutput")
        dbg_m = nc.dram_tensor("d_m", [2, C, QPC], f32, kind="ExternalOutput")
        dbg_gate = nc.dram_tensor("d_gate", [2, C, 1], f32, kind="ExternalOutput")
        dbg_red = nc.dram_tensor("d_red", [2, C, 2], f32, kind="ExternalOutput")

    cc_in = [nc.dram_tensor(f"cc_in{i}", [C, 2], f32, kind="Internal") for i in range(2)]
    cc_out = [nc.dram_tensor(f"cc_out{i}", [C, 2], f32, kind="Internal") for i in range(2)]
    groups = [[0, 1, 2, 3], [4, 5, 6, 7]]

    with TileContext(nc) as tc:
        import contextlib
        stack = contextlib.ExitStack()
        cpool = stack.enter_context(tc.tile_pool(name="const", bufs=1))
        spool = stack.enter_context(tc.tile_pool(name="scores", bufs=1 if debug else 2))
        tkpool = stack.enter_context(tc.tile_pool(name="topk", bufs=2))
        gpool = stack.enter_context(tc.tile_pool(name="gath", bufs=1 if debug else 2))
        mpool = stack.enter_context(tc.tile_pool(name="msum", bufs=1))
        dpool = stack.enter_context(tc.tile_pool(name="dram", bufs=1, space="DRAM"))
        pps = stack.enter_context(tc.tile_pool(name="pmm", bufs=2, space="PSUM"))
        pps_s = stack.enter_context(tc.tile_pool(name="psmall", bufs=1, space="PSUM"))

        # ---------------- loads ----------------
        X = cpool.tile([C, HW], f32, tag="X")
        Xq = cpool.tile([C, QPC], f32, tag="Xq")
        nc.sync.dma_start(out=X, in_=x_d[:, :])
        nc.sync.dma_start(out=Xq, in_=xq_d[:, :])
        rgbq = cpool.tile([4, QPC], f32, tag="rgbq")
        rgbk = cpool.tile([4, HW], f32, tag="rgbk")
        irq = cpool.tile([2, QPC], f32, tag="irq")
        irk = cpool.tile([2, HW], f32, tag="irk")
        nc.sync.dma_start(out=rgbq, in_=rgbq_d[:, :])
        nc.sync.dma_start(out=rgbk, in_=rgbk_d[:, :])
        nc.sync.dma_start(out=irq, in_=irq_d[:, :])
        nc.sync.dma_start(out=irk, in_=irk_d[:, :])
        wz = cpool.tile([C, 4 * C], f32, tag="wz")
        nc.sync.dma_start(out=wz, in_=wz_d[:, :])
        brgb = cpool.tile([C, 1], f32, tag="brgb")
        bir = cpool.tile([C, 1], f32, tag="bir")
        nc.sync.dma_start(out=brgb, in_=brgb_d[:, :])
        nc.sync.dma_start(out=bir, in_=bir_d[:, :])
        se1wa = cpool.tile([C, C // 16], f32, tag="se1wa")
        se1wb = cpool.tile([C, C // 16], f32, tag="se1wb")
        nc.sync.dma_start(out=se1wa, in_=se1w_d[0:C, :])
        nc.sync.dma_start(out=se1wb, in_=se1w_d[C:2 * C, :])
        se1b = cpool.tile([C // 16, 1], f32, tag="se1b")
        se2w = cpool.tile([C // 16, C], f32, tag="se2w")
        se2b = cpool.tile([C, 1], f32, tag="se2b")
        nc.sync.dma_start(out=se1b, in_=se1b_d[:, :])
        nc.sync.dma_start(out=se2w, in_=se2w_d[:, :])
        nc.sync.dma_start(out=se2b, in_=se2b_d[:, :])
        gam = cpool.tile([1, 1], f32, tag="gam")
        nc.sync.dma_start(out=gam, in_=gamma_d[:, :])
        ident = cpool.tile([C, C], f32, tag="ident")
        nc.sync.dma_start(out=ident, in_=nc.inline_tensor(np.eye(C, dtype=np.float32), "identc")[:, :])
        import ml_dtypes
        identb = cpool.tile([C, C], bf16, tag="identb")
        nc.sync.dma_start(out=identb, in_=nc.inline_tensor(np.eye(C, dtype=ml_dtypes.bfloat16), "identb")[:, :])

        if gmode == "fp32once":
            hGd = [dpool.tile([C, K * QPC], f32, tag=f"hGd{m}", name=f"hGd{m}")
                   for m in range(2)]
        if gmode == "bf16once":
            Xb = cpool.tile([C, HW], bf16, tag="Xb")
            nc.scalar.copy(Xb, X)
            hG = [cpool.tile([C, K * QPC], bf16, tag=f"hG{m}", name=f"hG{m}")
                  for m in range(2)]

        # ---------------- scores + top-16 ----------------
        # idxw[mod]: [128, QPC] int16, wrapped idx lists replicated per 16-part group
        idxw = [cpool.tile([C, QPC], i16, tag=f"idxw{m}", name=f"idxw{m}") for m in range(2)]
        idx16w = [cpool.tile([16, QPC], i16, tag=f"idx16w{m}", name=f"idx16w{m}") for m in range(2)]

        for m, (qf, kf, kdim) in enumerate([(rgbq, rgbk, 4), (irq, irk, 2)]):
            for qt in range(NQT):
                S = spool.tile([C, HW], f32, tag="S")
                for half in range(4):
                    ps = pps.tile([C, 1024], f32, tag="mm")
                    for j in range(2):
                        col = half * 1024 + j * 512
                        nc.tensor.matmul(
                            ps[:, j * 512:(j + 1) * 512],
                            qf[0:kdim, qt * 128:(qt + 1) * 128],
                            kf[0:kdim, col:col + 512],
                        )
                    nc.scalar.copy(S[:, half * 1024:(half + 1) * 1024], ps)
                if "notopk" in ablate:
                    continue
                # chunked top-8s
                pooled = tkpool.tile([C, NCHUNK * 8], f32, tag="pooled")
                for cch in range(NCHUNK):
                    nc.vector.max(out=pooled[:, cch * 8:(cch + 1) * 8],
                                  in_=S[:, cch * CHUNK:(cch + 1) * CHUNK])
                t8a = tkpool.tile([C, 8], f32, tag="t8a")
                t8b = tkpool.tile([C, 8], f32, tag="t8b")
                pooled2 = tkpool.tile([C, NCHUNK * 8], f32, tag="pooled2")
                nc.vector.max(out=t8a, in_=pooled)
                nc.vector.match_replace(out=pooled2, in_to_replace=t8a,
                                        in_values=pooled, imm_value=-3.0e38)
                nc.vector.max(out=t8b, in_=pooled2)
                idxq = tkpool.tile([C, 16], u16, tag="idxq")
                nc.vector.max_index(out=idxq[:, 0:8], in_max=t8a, in_values=S)
                nc.vector.max_index(out=idxq[:, 8:16], in_max=t8b, in_values=S)
                idxf = tkpool.tile([C, 16], f32, tag="idxf")
                nc.vector.tensor_copy(idxf, idxq)
                ptr = pps_s.tile([16, C], f32, tag="ptr")
                nc.tensor.transpose(ptr, idxf, ident[:, :])
                nc.vector.tensor_copy(idx16w[m][:, qt * 128:(qt + 1) * 128], ptr)
            if "notopk" in ablate:
                nc.vector.memset(idx16w[m], 0)
            # replicate [16, QPC] to all 8 groups
            for g in range(8):
                nc.sync.dma_start(out=idxw[m][g * 16:(g + 1) * 16, :], in_=idx16w[m])
            if debug:
                dif = spool.tile([16, QPC], f32, tag="dbgidx")
                nc.vector.tensor_copy(dif, idx16w[m])
                nc.sync.dma_start(out=dbg_idx[m], in_=dif)

        # ---------------- GNN iterations ----------------
        if gmode == "bf16once" and "nogather" not in ablate:
            for qt in range(NQT):
                isl = slice(qt * 128, (qt + 1) * 128)
                osl = slice(qt * 2048, (qt + 1) * 2048)
                nc.gpsimd.ap_gather(out_ap=hG[0][:, osl], in_ap=Xb, idxs_ap=idxw[0][:, isl],
                                    channels=C, num_elems=HW, d=1, num_idxs=2048)
                nc.gpsimd.ap_gather(out_ap=hG[1][:, osl], in_ap=Xb, idxs_ap=idxw[1][:, isl],
                                    channels=C, num_elems=HW, d=1, num_idxs=2048)
        gates = []
        wcur = wz
        for it in range(2):
            if it == 1:
                wdt = bf16 if gmode == "bf16once" else f32
                wscaled = cpool.tile([C, 4 * C], wdt, tag="wscaled")
                nc.scalar.activation(wscaled, wz, AF.Copy, scale=gates[0])
                wcur = wscaled
            elif gmode == "bf16once":
                wzb = cpool.tile([C, 4 * C], bf16, tag="wzb")
                nc.scalar.copy(wzb, wz)
                wcur = wzb

            m_br = [mpool.tile([C, QPC], f32, tag=f"m{b}", name=f"m{b}") for b in range(2)]
            if "nogather" in ablate:
                ga_st = mpool.tile([C, 2048], f32, tag="ga_st")
                gb_st = mpool.tile([C, 2048], f32, tag="gb_st")
                nc.vector.memset(ga_st, 0.125)
                nc.vector.memset(gb_st, 0.25)
            # hGr/hGi: gathered point features h[idx].T, [128 ch, 2048 (q,k)]
            # branch rgb: pre-max = (W1+W2).T hGr + (-W2).T hGi   (wz blocks 0,1)
            # branch ir:  pre-max = (U1+U2).T hGi + (-U2).T hGr   (wz blocks 2,3)
            for qt in range(NQT):
                if "nogather" in ablate:
                    ga, gb = ga_st, gb_st
                elif gmode == "bf16once":
                    osl = slice(qt * 2048, (qt + 1) * 2048)
                    ga = hG[0][:, osl]
                    gb = hG[1][:, osl]
                elif gmode == "fp32once":
                    osl = slice(qt * 2048, (qt + 1) * 2048)
                    ga = gpool.tile([C, 2048], f32, tag="ga")
                    gb = gpool.tile([C, 2048], f32, tag="gb")
                    if it == 0:
                        isl = slice(qt * 128, (qt + 1) * 128)
                        nc.gpsimd.ap_gather(out_ap=ga, in_ap=X, idxs_ap=idxw[0][:, isl],
                                            channels=C, num_elems=HW, d=1, num_idxs=2048)
                        nc.gpsimd.ap_gather(out_ap=gb, in_ap=X, idxs_ap=idxw[1][:, isl],
                                            channels=C, num_elems=HW, d=1, num_idxs=2048)
                        nc.sync.dma_start(out=hGd[0][:, osl], in_=ga)
                        nc.sync.dma_start(out=hGd[1][:, osl], in_=gb)
                    else:
                        nc.sync.dma_start(out=ga, in_=hGd[0][:, osl])
                        nc.sync.dma_start(out=gb, in_=hGd[1][:, osl])
                else:
                    ga = gpool.tile([C, 2048], f32, tag="ga")
                    gb = gpool.tile([C, 2048], f32, tag="gb")
                    isl = slice(qt * 128, (qt + 1) * 128)
                    nc.gpsimd.ap_gather(out_ap=ga, in_ap=X, idxs_ap=idxw[0][:, isl],
                                        channels=C, num_elems=HW, d=1, num_idxs=2048)
                    nc.gpsimd.ap_gather(out_ap=gb, in_ap=X, idxs_ap=idxw[1][:, isl],
                                        channels=C, num_elems=HW, d=1, num_idxs=2048)
                for b in range(2):
                    wa = wcur[:, (0 if b == 0 else 2) * C:(1 if b == 0 else 3) * C]
                    wb = wcur[:, (1 if b == 0 else 3) * C:(2 if b == 0 else 4) * C]
                    ra, rb = (ga, gb) if b == 0 else (gb, ga)
                    for half in range(2):
                        pd = pps.tile([C, 1024], f32, tag="mm")
                        for j in range(2):
                            sl = slice(j * 512, (j + 1) * 512)
                            gsl = slice(half * 1024 + j * 512, half * 1024 + (j + 1) * 512)
                            nc.tensor.matmul(pd[:, sl], wa, ra[:, gsl],
                                             start=True, stop=False)
                            nc.tensor.matmul(pd[:, sl], wb, rb[:, gsl],
                                             start=False, stop=True)
                        nc.vector.tensor_reduce(
                            out=m_br[b][:, qt * 128 + half * 64: qt * 128 + (half + 1) * 64],
                            in_=pd.rearrange("p (a b) -> p a b", b=K),
                            axis=AX.X, op=OP.max)
            packed = mpool.tile([C, 2], f32, tag="packed")
            for b, bias in ((0, brgb), (1, bir)):
                mr = mpool.tile([C, QPC], f32, tag=f"mr{b}")
                nc.scalar.activation(mr, m_br[b], AF.Relu, bias=bias,
                                     accum_out=packed[:, b:b + 1])
                if debug:
                    nc.sync.dma_start(out=dbg_m[b], in_=mr)
            # allreduce partial sums across the 4 cores of this sample
            red = mpool.tile([C, 2], f32, tag="red")
            if sim:
                # TimelineSim can't run collectives; keep timing-equivalent DMAs
                nc.sync.dma_start(out=cc_in[it][:, :], in_=packed)
                nc.sync.dma_start(out=red, in_=cc_in[it][:, :])
            else:
                nc.sync.dma_start(out=cc_in[it][:, :], in_=packed)
                nc.gpsimd.collective_compute(
                    "AllReduce", mybir.AluOpType.add, replica_groups=groups,
                    ins=[cc_in[it][:, :]], outs=[cc_out[it][:, :]])
                nc.sync.dma_start(out=red, in_=cc_out[it][:, :])
            if debug:
                nc.sync.dma_start(out=dbg_red[it], in_=red)
            # SE gate
            p8 = pps_s.tile([C // 16, 1], f32, tag="p8")
            nc.tensor.matmul(p8, se1wa, red[:, 0:1], start=True, stop=False)
            nc.tensor.matmul(p8, se1wb, red[:, 1:2], start=False, stop=True)
            fc1 = mpool.tile([C // 16, 1], f32, tag="fc1")
            nc.scalar.activation(fc1, p8, AF.Relu, bias=se1b, scale=1.0 / HW)
            pg = pps_s.tile([C, 1], f32, tag="pg")
            nc.tensor.matmul(pg, se2w, fc1)
            gate = mpool.tile([C, 1], f32, tag=f"gate{it}")
            nc.scalar.activation(gate, pg, AF.Sigmoid, bias=se2b)
            gates.append(gate)
            if debug:
                nc.sync.dma_start(out=dbg_gate[it], in_=gate)

        # ---------------- final output ----------------
        gamb = mpool.tile([C, 1], f32, tag="gamb")
        nc.gpsimd.partition_broadcast(gamb, gam, channels=C)
        sfin = mpool.tile([C, 1], f32, tag="sfin")
        nc.vector.tensor_mul(sfin, gates[0], gates[1])
        nc.vector.tensor_mul(sfin, sfin, gamb)
        nc.vector.tensor_scalar_add(sfin, sfin, 1.0)
        outt = mpool.tile([C, QPC], f32, tag="mr0")
        nc.scalar.activation(outt, Xq, AF.Relu, scale=sfin)
        nc.sync.dma_start(out=out_d[:, :], in_=outt)
        stack.close()

    nc.compile()
    return nc


def _get_nc(debug=False):
    key = ("nc", debug)
    if key not in _CACHE:
        _CACHE[key] = _build(debug)
    return _CACHE[key]


def _shard_inputs(cnn_encoder_output, rgb, ir, rgb_g_w, rgb_g_b, ir_g_w, ir_g_b,
                  se1_w, se1_b, se2_w, se2_b, gamma):
    x = np.asarray(cnn_encoder_output, np.float32)
    rgbf = np.asarray(rgb, np.float32).reshape(N, 3, HW)
    irf = np.asarray(ir, np.float32).reshape(N, 1, HW)
    W1 = np.asarray(rgb_g_w, np.float32)[:C]
    W2 = np.asarray(rgb_g_w, np.float32)[C:]
    U1 = np.asarray(ir_g_w, np.float32)[:C]
    U2 = np.asarray(ir_g_w, np.float32)[C:]
    wz = np.concatenate([W1 + W2, -W2, U1 + U2, -U2], axis=1)  # [z1 | z2n | z3 | z4n]

    in_maps = []
    for core in range(NCORES):
        n, g = core // 4, core % 4
        sl = slice(g * QPC, (g + 1) * QPC)
        Xn = np.ascontiguousarray(x[n].reshape(C, HW))
        sq_rgb = (rgbf[n] ** 2).sum(axis=0)
        sq_ir = (irf[n] ** 2).sum(axis=0)
        # scores S' = 2<f_q, f_j> - sq_j  (descending S' == ascending distance)
        rgbq = np.concatenate([2.0 * rgbf[n][:, sl], -np.ones((1, QPC), np.float32)], 0)
        rgbk = np.concatenate([rgbf[n], sq_rgb[None]], 0)
        irq = np.concatenate([2.0 * irf[n][:, sl], -np.ones((1, QPC), np.float32)], 0)
        irk = np.concatenate([irf[n], sq_ir[None]], 0)
        in_maps.append({
            "x": Xn,
            "xq": np.ascontiguousarray(Xn[:, sl]),
            "rgbq": np.ascontiguousarray(rgbq, np.float32),
            "rgbk": np.ascontiguousarray(rgbk, np.float32),
            "irq": np.ascontiguousarray(irq, np.float32),
            "irk": np.ascontiguousarray(irk, np.float32),
            "wz": np.ascontiguousarray(wz),
            "brgb": np.asarray(rgb_g_b, np.float32).reshape(C, 1),
            "bir": np.asarray(ir_g_b, np.float32).reshape(C, 1),
            "se1w": np.ascontiguousarray(np.asarray(se1_w, np.float32)),
            "se1b": np.asarray(se1_b, np.float32).reshape(C // 16, 1),
            "se2w": np.ascontiguousarray(np.asarray(se2_w, np.float32)),
            "se2b": np.asarray(se2_b, np.float32).reshape(C, 1),
            "gammav": np.asarray(gamma, np.float32).reshape(1, 1),
        })
    return in_maps


def kernel(cnn_encoder_output, rgb, ir, rgb_g_w, rgb_g_b, ir_g_w, ir_g_b,
           se1_w, se1_b, se2_w, se2_b, gamma, gnn_iterations, k,
           _debug=False, _trace=False):
    from concourse.bass_utils import run_bass_kernel_spmd

    assert int(gnn_iterations) == 2 and int(k) == 16
    nc = _get_nc(_debug)
    in_maps = _shard_inputs(cnn_encoder_output, rgb, ir, rgb_g_w, rgb_g_b,
                            ir_g_w, ir_g_b, se1_w, se1_b, se2_w, se2_b, gamma)
    kw = {}
    if _trace:
        kw = dict(trace=True)
    res = run_bass_kernel_spmd(nc, in_maps, core_ids=list(range(NCORES)), **kw)
    _CACHE["last_result"] = res
    out = np.empty((N, C, H, W), np.float32)
    for core in range(NCORES):
        n, g = core // 4, core % 4
        out[n].reshape(C, HW)[:, g * QPC:(g + 1) * QPC] = res.results[core]["out"]
    return out



# revision 5
# speedup vs baseline: 3.0674x; 3.0674x over previous
"""Trainium2 Bass kernel for nn_EnetGnn (gnn_message_passing).

Self-contained: accepts FULL inputs, shards across 8 NeuronCores internally
(core c -> sample c//4, query-quarter c%4), returns the FULL output.

Math (per sample, h = x reshaped [HW, C], X = h.T [C, HW]):
  rgb_idx/ir_idx = top-16 smallest-distance neighbors per point (kNN on the
  3-ch rgb / 1-ch ir pixel features).  The reference MLP on gathered pairs
  factors through the gather:
    rf @ Wg = gather_rgb(h @ (W1+W2)) - gather_ir(h @ W2)
  and relu/bias commute with the max over k, so per branch:
    m = relu(b + max_k( Z_a[idx_a] + Z_bneg[idx_b] ))
  The SE gate is per-channel, so h_{t+1} = gate_t * h_t and iteration 2 only
  rescales the projection weights' rows by gate_1.  Final output:
    out = relu((1 + gamma * g1 * g2) * x).

v2: neighbor features are gathered ONCE (indices are iteration-invariant)
via DMA-gather (transpose mode) from an HBM-resident bf16 h=[HW, C] copy --
each index pulls one contiguous 256B channel-row, landing channels-across-
partitions directly.  All big matmuls run in bf16 (fp32 LOW_HIGH mode is
2x slower); PSUM accumulation stays fp32.
"""

import numpy as np

N, C, H, W = 2, 128, 64, 64
HW = H * W            # 4096
QPC = HW // 4         # queries per core = 1024
K = 16
NCORES = 8
NQT = QPC // 128      # query tiles per core = 8
CHUNK = 256           # max8 chunk size
NCHUNK = HW // CHUNK  # 16

_CACHE = {}


def _build():
    import concourse.bacc as bacc
    import concourse.mybir as mybir
    from concourse.tile import TileContext
    import ml_dtypes

    f32 = mybir.dt.float32
    bf16 = mybir.dt.bfloat16
    i16 = mybir.dt.int16
    u16 = mybir.dt.uint16
    AF = mybir.ActivationFunctionType
    OP = mybir.AluOpType
    AX = mybir.AxisListType

    nc = bacc.Bacc("TRN2", target_bir_lowering=False, debug=False, num_devices=NCORES)

    # ---------------- I/O ----------------
    xt_d = nc.dram_tensor("xt", [HW, C], bf16, kind="ExternalInput")
    xq_d = nc.dram_tensor("xq", [C, QPC], f32, kind="ExternalInput")
    rgbq_d = nc.dram_tensor("rgbq", [4, QPC], bf16, kind="ExternalInput")
    rgbk_d = nc.dram_tensor("rgbk", [4, HW], bf16, kind="ExternalInput")
    irq_d = nc.dram_tensor("irq", [2, QPC], bf16, kind="ExternalInput")
    irk_d = nc.dram_tensor("irk", [2, HW], bf16, kind="ExternalInput")
    wz_d = nc.dram_tensor("wz", [C, 4 * C], f32, kind="ExternalInput")
    brgb_d = nc.dram_tensor("brgb", [C, 1], f32, kind="ExternalInput")
    bir_d = nc.dram_tensor("bir", [C, 1], f32, kind="ExternalInput")
    se1w_d = nc.dram_tensor("se1w", [2 * C, C // 16], f32, kind="ExternalInput")
    se1b_d = nc.dram_tensor("se1b", [C // 16, 1], f32, kind="ExternalInput")
    se2w_d = nc.dram_tensor("se2w", [C // 16, C], f32, kind="ExternalInput")
    se2b_d = nc.dram_tensor("se2b", [C, 1], f32, kind="ExternalInput")
    gamma_d = nc.dram_tensor("gammav", [1, 1], f32, kind="ExternalInput")
    out_d = nc.dram_tensor("out", [C, QPC], f32, kind="ExternalOutput")

    cc_in = [nc.dram_tensor(f"cc_in{i}", [C, 2], f32, kind="Internal") for i in range(2)]
    cc_out = [nc.dram_tensor(f"cc_out{i}", [C, 2], f32, kind="Internal") for i in range(2)]
    groups = [[0, 1, 2, 3], [4, 5, 6, 7]]

    with TileContext(nc) as tc:
        import contextlib
        stack = contextlib.ExitStack()
        cpool = stack.enter_context(tc.tile_pool(name="const", bufs=1))
        spool = stack.enter_context(tc.tile_pool(name="scores", bufs=2))
        tkpool = stack.enter_context(tc.tile_pool(name="topk", bufs=2))
        mpool = stack.enter_context(tc.tile_pool(name="msum", bufs=1))
        pps = stack.enter_context(tc.tile_pool(name="pmm", bufs=2, space="PSUM"))
        pps_s = stack.enter_context(tc.tile_pool(name="psmall", bufs=1, space="PSUM"))

        # ---------------- loads ----------------
        Xq = cpool.tile([C, QPC], f32, tag="Xq")
        nc.sync.dma_start(out=Xq, in_=xq_d[:, :])
        rgbq = cpool.tile([4, QPC], bf16, tag="rgbq")
        rgbk = cpool.tile([4, HW], bf16, tag="rgbk")
        irq = cpool.tile([2, QPC], bf16, tag="irq")
        irk = cpool.tile([2, HW], bf16, tag="irk")
        nc.sync.dma_start(out=rgbq, in_=rgbq_d[:, :])
        nc.sync.dma_start(out=rgbk, in_=rgbk_d[:, :])
        nc.sync.dma_start(out=irq, in_=irq_d[:, :])
        nc.sync.dma_start(out=irk, in_=irk_d[:, :])
        wz = cpool.tile([C, 4 * C], f32, tag="wz")
        nc.sync.dma_start(out=wz, in_=wz_d[:, :])
        brgb = cpool.tile([C, 1], f32, tag="brgb")
        bir = cpool.tile([C, 1], f32, tag="bir")
        nc.sync.dma_start(out=brgb, in_=brgb_d[:, :])
        nc.sync.dma_start(out=bir, in_=bir_d[:, :])
        se1wa = cpool.tile([C, C // 16], f32, tag="se1wa")
        se1wb = cpool.tile([C, C // 16], f32, tag="se1wb")
        nc.sync.dma_start(out=se1wa, in_=se1w_d[0:C, :])
        nc.sync.dma_start(out=se1wb, in_=se1w_d[C:2 * C, :])
        se1b = cpool.tile([C // 16, 1], f32, tag="se1b")
        se2w = cpool.tile([C // 16, C], f32, tag="se2w")
        se2b = cpool.tile([C, 1], f32, tag="se2b")
        nc.sync.dma_start(out=se1b, in_=se1b_d[:, :])
        nc.sync.dma_start(out=se2w, in_=se2w_d[:, :])
        nc.sync.dma_start(out=se2b, in_=se2b_d[:, :])
        gam = cpool.tile([1, 1], f32, tag="gam")
        nc.sync.dma_start(out=gam, in_=gamma_d[:, :])
        ident = cpool.tile([C, C], f32, tag="ident")
        nc.sync.dma_start(out=ident, in_=nc.inline_tensor(np.eye(C, dtype=np.float32), "identc")[:, :])

        # bf16 weights for iteration 1
        wzb = cpool.tile([C, 4 * C], bf16, tag="wzb")
        nc.scalar.copy(wzb, wz)

        # gathered neighbor features, both modalities, all 8 query tiles
        hG = [cpool.tile([C, 1, K * QPC], bf16, tag=f"hG{m}", name=f"hG{m}")
              for m in range(2)]
        # idxw[mod]: [128, QPC] int16, wrapped idx lists replicated per 16-part group
        idxw = [cpool.tile([C, QPC], i16, tag=f"idxw{m}", name=f"idxw{m}") for m in range(2)]

        m_br = [mpool.tile([C, QPC], f32, tag=f"m{b}", name=f"m{b}") for b in range(2)]

        # ---------------- phase A: scores + top-16 + gather + it-1 MLP ----------------
        def mlp_qt(qt, wcur, target):
            """pre-max = wa.T ga + wb.T gb per branch; max over k=16 -> target[b]."""
            ga = hG[0][:, 0, qt * 2048:(qt + 1) * 2048]
            gb = hG[1][:, 0, qt * 2048:(qt + 1) * 2048]
            for b in range(2):
                wa = wcur[:, (0 if b == 0 else 2) * C:(1 if b == 0 else 3) * C]
                wb = wcur[:, (1 if b == 0 else 3) * C:(2 if b == 0 else 4) * C]
                ra, rb = (ga, gb) if b == 0 else (gb, ga)
                for half in range(2):
                    pd = pps.tile([C, 1024], f32, tag="mm")
                    for j in range(2):
                        sl = slice(j * 512, (j + 1) * 512)
                        gsl = slice(half * 1024 + j * 512, half * 1024 + (j + 1) * 512)
                        nc.tensor.matmul(pd[:, sl], wa, ra[:, gsl],
                                         start=True, stop=False)
                        nc.tensor.matmul(pd[:, sl], wb, rb[:, gsl],
                                         start=False, stop=True)
                    nc.vector.tensor_reduce(
                        out=target[b][:, qt * 128 + half * 64: qt * 128 + (half + 1) * 64],
                        in_=pd.rearrange("p (a b) -> p a b", b=K),
                        axis=AX.X, op=OP.max)

        for qt in range(NQT):
            for m, (qf, kf, kdim) in enumerate([(rgbq, rgbk, 4), (irq, irk, 2)]):
                S = spool.tile([C, HW], f32, tag="S")
                for half in range(4):
                    ps = pps.tile([C, 1024], f32, tag="mm")
                    for j in range(2):
                        col = half * 1024 + j * 512
                        nc.tensor.matmul(
                            ps[:, j * 512:(j + 1) * 512],
                            qf[0:kdim, qt * 128:(qt + 1) * 128],
                            kf[0:kdim, col:col + 512],
                        )
                    nc.scalar.copy(S[:, half * 1024:(half + 1) * 1024], ps)
                # chunked top-8s
                pooled = tkpool.tile([C, NCHUNK * 8], f32, tag="pooled")
                for cch in range(NCHUNK):
                    nc.vector.max(out=pooled[:, cch * 8:(cch + 1) * 8],
                                  in_=S[:, cch * CHUNK:(cch + 1) * CHUNK])
                t8a = tkpool.tile([C, 8], f32, tag="t8a")
                t8b = tkpool.tile([C, 8], f32, tag="t8b")
                pooled2 = tkpool.tile([C, NCHUNK * 8], f32, tag="pooled2")
                nc.vector.max(out=t8a, in_=pooled)
                nc.vector.match_replace(out=pooled2, in_to_replace=t8a,
                                        in_values=pooled, imm_value=-3.0e38)
                nc.vector.max(out=t8b, in_=pooled2)
                idxq = tkpool.tile([C, 16], u16, tag="idxq")
                nc.vector.max_index(out=idxq[:, 0:8], in_max=t8a, in_values=S)
                nc.vector.max_index(out=idxq[:, 8:16], in_max=t8b, in_values=S)
                idxf = tkpool.tile([C, 16], f32, tag="idxf")
                nc.vector.tensor_copy(idxf, idxq)
                # widen to 8 replicas, then one PE transpose yields the
                # "wrapped in 16 partitions, replicated per group" layout
                # directly (avoids SBUF->SBUF DMAs, which HW-deadlock
                # against in-flight xbar-transpose gathers)
                idxfR = tkpool.tile([C, 128], f32, tag="idxfR")
                for g in range(8):
                    nc.vector.tensor_copy(idxfR[:, g * 16:(g + 1) * 16], idxf)
                ptr = pps_s.tile([128, C], f32, tag="ptr")
                nc.tensor.transpose(ptr, idxfR, ident[:, :])
                nc.vector.tensor_copy(idxw[m][:, qt * 128:(qt + 1) * 128], ptr)
                # gather neighbor channel-rows from HBM (256B each), bf16,
                # landing channels-across-partitions: hG[:, 0, i] = h[idx_i, :]
                nc.gpsimd.dma_gather(
                    out_ap=hG[m][:, :, qt * 2048:(qt + 1) * 2048],
                    in_ap=xt_d[:, :],
                    idxs_ap=idxw[m][:, qt * 128:(qt + 1) * 128],
                    num_idxs=2048, num_idxs_reg=2048,
                    elem_size=C, transpose=True,
                    # >64 descriptors in one packet wedges the SDMA engine
                    single_packet=False)
            mlp_qt(qt, wzb, m_br)

        # ---------------- SE gate / iteration boundary ----------------
        gates = []
        m_br2 = [mpool.tile([C, QPC], f32, tag=f"m2_{b}", name=f"m2_{b}") for b in range(2)]
        for it in range(2):
            cur = m_br if it == 0 else m_br2
            packed = mpool.tile([C, 2], f32, tag=f"packed{it}")
            for b, bias in ((0, brgb), (1, bir)):
                mr = mpool.tile([C, QPC], f32, tag=f"mr{b}")
                nc.scalar.activation(mr, cur[b], AF.Relu, bias=bias,
                                     accum_out=packed[:, b:b + 1])
            # allreduce partial sums across the 4 cores of this sample
            red = mpool.tile([C, 2], f32, tag=f"red{it}")
            nc.sync.dma_start(out=cc_in[it][:, :], in_=packed)
            nc.gpsimd.collective_compute(
                "AllReduce", mybir.AluOpType.add, replica_groups=groups,
                ins=[cc_in[it][:, :]], outs=[cc_out[it][:, :]])
            nc.sync.dma_start(out=red, in_=cc_out[it][:, :])
            # SE gate
            p8 = pps_s.tile([C // 16, 1], f32, tag="p8")
            nc.tensor.matmul(p8, se1wa, red[:, 0:1], start=True, stop=False)
            nc.tensor.matmul(p8, se1wb, red[:, 1:2], start=False, stop=True)
            fc1 = mpool.tile([C // 16, 1], f32, tag="fc1")
            nc.scalar.activation(fc1, p8, AF.Relu, bias=se1b, scale=1.0 / HW)
            pg = pps_s.tile([C, 1], f32, tag="pg")
            nc.tensor.matmul(pg, se2w, fc1)
            gate = mpool.tile([C, 1], f32, tag=f"gate{it}")
            nc.scalar.activation(gate, pg, AF.Sigmoid, bias=se2b)
            gates.append(gate)
            if it == 0:
                # iteration 2: rescale weight rows by gate_1, rerun MLP
                wscaled = cpool.tile([C, 4 * C], bf16, tag="wscaled")
                nc.scalar.activation(wscaled, wz, AF.Copy, scale=gates[0])
                for qt in range(NQT):
                    mlp_qt(qt, wscaled, m_br2)

        # ---------------- final output ----------------
        gamb = mpool.tile([C, 1], f32, tag="gamb")
        nc.gpsimd.partition_broadcast(gamb, gam, channels=C)
        sfin = mpool.tile([C, 1], f32, tag="sfin")
        nc.vector.tensor_mul(sfin, gates[0], gates[1])
        nc.vector.tensor_mul(sfin, sfin, gamb)
        nc.vector.tensor_scalar_add(sfin, sfin, 1.0)
        outt = mpool.tile([C, QPC], f32, tag="outt")
        nc.scalar.activation(outt, Xq, AF.Relu, scale=sfin)
        nc.sync.dma_start(out=out_d[:, :], in_=outt)
        stack.close()

    nc.compile()
    return nc


def _get_nc():
    if "nc" not in _CACHE:
        _CACHE["nc"] = _build()
    return _CACHE["nc"]


def _shard_inputs(cnn_encoder_output, rgb, ir, rgb_g_w, rgb_g_b, ir_g_w, ir_g_b,
                  se1_w, se1_b, se2_w, se2_b, gamma):
    import ml_dtypes
    bf16 = ml_dtypes.bfloat16
    x = np.asarray(cnn_encoder_output, np.float32)
    rgbf = np.asarray(rgb, np.float32).reshape(N, 3, HW)
    irf = np.asarray(ir, np.float32).reshape(N, 1, HW)
    W1 = np.asarray(rgb_g_w, np.float32)[:C]
    W2 = np.asarray(rgb_g_w, np.float32)[C:]
    U1 = np.asarray(ir_g_w, np.float32)[:C]
    U2 = np.asarray(ir_g_w, np.float32)[C:]
    wz = np.concatenate([W1 + W2, -W2, U1 + U2, -U2], axis=1)  # [z1 | z2n | z3 | z4n]

    in_maps = []
    for core in range(NCORES):
        n, g = core // 4, core % 4
        sl = slice(g * QPC, (g + 1) * QPC)
        Xn = np.ascontiguousarray(x[n].reshape(C, HW))
        sq_rgb = (rgbf[n] ** 2).sum(axis=0)
        sq_ir = (irf[n] ** 2).sum(axis=0)
        # scores S' = 2<f_q, f_j> - sq_j  (descending S' == ascending distance)
        rgbq = np.concatenate([2.0 * rgbf[n][:, sl], -np.ones((1, QPC), np.float32)], 0)
        rgbk = np.concatenate([rgbf[n], sq_rgb[None]], 0)
        irq = np.concatenate([2.0 * irf[n][:, sl], -np.ones((1, QPC), np.float32)], 0)
        irk = np.concatenate([irf[n], sq_ir[None]], 0)
        in_maps.append({
            "xt": np.ascontiguousarray(Xn.T).astype(bf16),
            "xq": np.ascontiguousarray(Xn[:, sl]),
            "rgbq": np.ascontiguousarray(rgbq).astype(bf16),
            "rgbk": np.ascontiguousarray(rgbk).astype(bf16),
            "irq": np.ascontiguousarray(irq).astype(bf16),
            "irk": np.ascontiguousarray(irk).astype(bf16),
            "wz": np.ascontiguousarray(wz),
            "brgb": np.asarray(rgb_g_b, np.float32).reshape(C, 1),
            "bir": np.asarray(ir_g_b, np.float32).reshape(C, 1),
            "se1w": np.ascontiguousarray(np.asarray(se1_w, np.float32)),
            "se1b": np.asarray(se1_b, np.float32).reshape(C // 16, 1),
            "se2w": np.ascontiguousarray(np.asarray(se2_w, np.float32)),
            "se2b": np.asarray(se2_b, np.float32).reshape(C, 1),
            "gammav": np.asarray(gamma, np.float32).reshape(1, 1),
        })
    return in_maps


def kernel(cnn_encoder_output, rgb, ir, rgb_g_w, rgb_g_b, ir_g_w, ir_g_b,
           se1_w, se1_b, se2_w, se2_b, gamma, gnn_iterations, k,
           _debug=False, _trace=False):
    from concourse.bass_utils import run_bass_kernel_spmd

    assert int(gnn_iterations) == 2 and int(k) == 16
    nc = _get_nc()
    in_maps = _shard_inputs(cnn_encoder_output, rgb, ir, rgb_g_w, rgb_g_b,
                            ir_g_w, ir_g_b, se1_w, se1_b, se2_w, se2_b, gamma)
    kw = {}
    if _trace:
        kw = dict(trace=True)
    res = run_bass_kernel_spmd(nc, in_maps, core_ids=list(range(NCORES)), **kw)
    _CACHE["last_result"] = res
    out = np.empty((N, C, H, W), np.float32)
    for core in range(NCORES):
        n, g = core // 4, core % 4
        out[n].reshape(C, HW)[:, g * QPC:(g + 1) * QPC] = res.results[core]["out"]
    return out


# revision 11
# speedup vs baseline: 4.5166x; 1.4724x over previous
"""Trainium2 Bass kernel for nn_EnetGnn (gnn_message_passing).

Self-contained: accepts FULL inputs, shards across 8 NeuronCores internally
(core c -> sample c//4, query-quarter c%4), returns the FULL output.

Math (per sample, h = x reshaped [HW, C], X = h.T [C, HW]):
  rgb_idx/ir_idx = top-16 smallest-distance neighbors per point (kNN on the
  3-ch rgb / 1-ch ir pixel features).  The reference MLP on gathered pairs
  factors through the gather:
    rf @ Wg = gather_rgb(h @ (W1+W2)) - gather_ir(h @ W2)
  and relu/bias commute with the max over k, so per branch:
    m = relu(b + max_k( Z_a[idx_a] + Z_bneg[idx_b] ))
  The SE gate is per-channel, so h_{t+1} = gate_t * h_t and iteration 2 only
  rescales the projection weights' rows by gate_1.  Final output:
    out = relu((1 + gamma * g1 * g2) * x).

v2: neighbor features are gathered ONCE (indices are iteration-invariant)
via DMA-gather (transpose mode) from an HBM-resident bf16 h=[HW, C] copy --
each index pulls one contiguous 256B channel-row, landing channels-across-
partitions directly.  All big matmuls run in bf16 (fp32 LOW_HIGH mode is
2x slower); PSUM accumulation stays fp32.
"""

import numpy as np

N, C, H, W = 2, 128, 64, 64
HW = H * W            # 4096
QPC = HW // 4         # queries per core = 1024
K = 16
NCORES = 8
NQT = QPC // 128      # query tiles per core = 8
CHUNK = 512           # max8 chunk size
NCHUNK = HW // CHUNK  # 8

_CACHE = {}


def _build():
    import concourse.bacc as bacc
    import concourse.mybir as mybir
    from concourse.tile import TileContext
    import ml_dtypes

    f32 = mybir.dt.float32
    bf16 = mybir.dt.bfloat16
    i16 = mybir.dt.int16
    u16 = mybir.dt.uint16
    AF = mybir.ActivationFunctionType
    OP = mybir.AluOpType
    AX = mybir.AxisListType

    nc = bacc.Bacc("TRN2", target_bir_lowering=False, debug=False, num_devices=NCORES)

    # ---------------- I/O ----------------
    xt_d = nc.dram_tensor("xt", [HW, C], bf16, kind="ExternalInput")
    xq_d = nc.dram_tensor("xq", [C, QPC], f32, kind="ExternalInput")
    rgbq_d = nc.dram_tensor("rgbq", [4, QPC], bf16, kind="ExternalInput")
    rgbk_d = nc.dram_tensor("rgbk", [4, HW], bf16, kind="ExternalInput")
    irq_d = nc.dram_tensor("irq", [2, QPC], bf16, kind="ExternalInput")
    irk_d = nc.dram_tensor("irk", [2, HW], bf16, kind="ExternalInput")
    wz_d = nc.dram_tensor("wz", [C, 4 * C], f32, kind="ExternalInput")
    brgb_d = nc.dram_tensor("brgb", [C, 1], f32, kind="ExternalInput")
    bir_d = nc.dram_tensor("bir", [C, 1], f32, kind="ExternalInput")
    se1w_d = nc.dram_tensor("se1w", [2 * C, C // 16], f32, kind="ExternalInput")
    se1b_d = nc.dram_tensor("se1b", [C // 16, 1], f32, kind="ExternalInput")
    se2w_d = nc.dram_tensor("se2w", [C // 16, C], f32, kind="ExternalInput")
    se2b_d = nc.dram_tensor("se2b", [C, 1], f32, kind="ExternalInput")
    gamma_d = nc.dram_tensor("gammav", [1, 1], f32, kind="ExternalInput")
    out_d = nc.dram_tensor("out", [C, QPC], f32, kind="ExternalOutput")

    cc_in = [nc.dram_tensor(f"cc_in{i}", [C, 2], f32, kind="Internal") for i in range(2)]
    cc_out = [nc.dram_tensor(f"cc_out{i}", [C, 2], f32, kind="Internal") for i in range(2)]
    groups = [[0, 1, 2, 3], [4, 5, 6, 7]]

    with TileContext(nc) as tc:
        import contextlib
        stack = contextlib.ExitStack()
        cpool = stack.enter_context(tc.tile_pool(name="const", bufs=1))
        spool = stack.enter_context(tc.tile_pool(name="scores", bufs=2))
        tkpool = stack.enter_context(tc.tile_pool(name="topk", bufs=2))
        mpool = stack.enter_context(tc.tile_pool(name="msum", bufs=1))
        pps = stack.enter_context(tc.tile_pool(name="pmm", bufs=2, space="PSUM"))
        pps_s = stack.enter_context(tc.tile_pool(name="psmall", bufs=1, space="PSUM"))
        pps_t = stack.enter_context(tc.tile_pool(name="ptrp", bufs=2, space="PSUM"))

        # ---------------- loads ----------------
        Xq = cpool.tile([C, QPC], f32, tag="Xq")
        nc.sync.dma_start(out=Xq, in_=xq_d[:, :])
        rgbq = cpool.tile([4, QPC], bf16, tag="rgbq")
        rgbk = cpool.tile([4, HW], bf16, tag="rgbk")
        irq = cpool.tile([2, QPC], bf16, tag="irq")
        irk = cpool.tile([2, HW], bf16, tag="irk")
        nc.sync.dma_start(out=rgbq, in_=rgbq_d[:, :])
        nc.sync.dma_start(out=rgbk, in_=rgbk_d[:, :])
        nc.sync.dma_start(out=irq, in_=irq_d[:, :])
        nc.sync.dma_start(out=irk, in_=irk_d[:, :])
        wz = cpool.tile([C, 4 * C], f32, tag="wz")
        nc.sync.dma_start(out=wz, in_=wz_d[:, :])
        brgb = cpool.tile([C, 1], f32, tag="brgb")
        bir = cpool.tile([C, 1], f32, tag="bir")
        nc.sync.dma_start(out=brgb, in_=brgb_d[:, :])
        nc.sync.dma_start(out=bir, in_=bir_d[:, :])
        se1wa = cpool.tile([C, C // 16], f32, tag="se1wa")
        se1wb = cpool.tile([C, C // 16], f32, tag="se1wb")
        nc.sync.dma_start(out=se1wa, in_=se1w_d[0:C, :])
        nc.sync.dma_start(out=se1wb, in_=se1w_d[C:2 * C, :])
        se1b = cpool.tile([C // 16, 1], f32, tag="se1b")
        se2w = cpool.tile([C // 16, C], f32, tag="se2w")
        se2b = cpool.tile([C, 1], f32, tag="se2b")
        nc.sync.dma_start(out=se1b, in_=se1b_d[:, :])
        nc.sync.dma_start(out=se2w, in_=se2w_d[:, :])
        nc.sync.dma_start(out=se2b, in_=se2b_d[:, :])
        gam = cpool.tile([1, 1], f32, tag="gam")
        nc.sync.dma_start(out=gam, in_=gamma_d[:, :])
        ident = cpool.tile([C, C], f32, tag="ident")
        nc.sync.dma_start(out=ident, in_=nc.inline_tensor(np.eye(C, dtype=np.float32), "identc")[:, :])

        # bf16 weights for iteration 1
        wzb = cpool.tile([C, 4 * C], bf16, tag="wzb")
        nc.scalar.copy(wzb, wz)

        # gathered neighbor features, both modalities, all 8 query tiles
        hG = [cpool.tile([C, 1, K * QPC], bf16, tag=f"hG{m}", name=f"hG{m}")
              for m in range(2)]
        # idxw[mod]: [128, QPC] int16, wrapped idx lists replicated per 16-part group
        idxw = [cpool.tile([C, QPC], i16, tag=f"idxw{m}", name=f"idxw{m}") for m in range(2)]

        m_br = [mpool.tile([C, QPC], f32, tag=f"m{b}", name=f"m{b}") for b in range(2)]

        # ---------------- phase A: scores + top-16 + gather + it-1 MLP ----------------
        def mlp_qt(qt, wcur, target):
            """pre-max = wa.T ga + wb.T gb per branch; max over k=16 -> target[b]."""
            ga = hG[0][:, 0, qt * 2048:(qt + 1) * 2048]
            gb = hG[1][:, 0, qt * 2048:(qt + 1) * 2048]
            for b in range(2):
                wa = wcur[:, (0 if b == 0 else 2) * C:(1 if b == 0 else 3) * C]
                wb = wcur[:, (1 if b == 0 else 3) * C:(2 if b == 0 else 4) * C]
                ra, rb = (ga, gb) if b == 0 else (gb, ga)
                for half in range(2):
                    pd = pps.tile([C, 1024], f32, tag="mm")
                    for j in range(2):
                        sl = slice(j * 512, (j + 1) * 512)
                        gsl = slice(half * 1024 + j * 512, half * 1024 + (j + 1) * 512)
                        nc.tensor.matmul(pd[:, sl], wa, ra[:, gsl],
                                         start=True, stop=False)
                        nc.tensor.matmul(pd[:, sl], wb, rb[:, gsl],
                                         start=False, stop=True)
                    nc.vector.tensor_reduce(
                        out=target[b][:, qt * 128 + half * 64: qt * 128 + (half + 1) * 64],
                        in_=pd.rearrange("p (a b) -> p a b", b=K),
                        axis=AX.X, op=OP.max)

        for qt in range(NQT):
            for m, (qf, kf, kdim) in enumerate([(rgbq, rgbk, 4), (irq, irk, 2)]):
                S = spool.tile([C, HW], f32, tag="S")
                for half in range(4):
                    ps = pps.tile([C, 1024], f32, tag="mm")
                    for j in range(2):
                        col = half * 1024 + j * 512
                        nc.tensor.matmul(
                            ps[:, j * 512:(j + 1) * 512],
                            qf[0:kdim, qt * 128:(qt + 1) * 128],
                            kf[0:kdim, col:col + 512],
                        )
                    nc.scalar.copy(S[:, half * 1024:(half + 1) * 1024], ps)
                # chunked top-8s
                pooled = tkpool.tile([C, NCHUNK * 8], f32, tag="pooled")
                for cch in range(NCHUNK):
                    nc.vector.max(out=pooled[:, cch * 8:(cch + 1) * 8],
                                  in_=S[:, cch * CHUNK:(cch + 1) * CHUNK])
                t8a = tkpool.tile([C, 8], f32, tag="t8a")
                t8b = tkpool.tile([C, 8], f32, tag="t8b")
                pooled2 = tkpool.tile([C, NCHUNK * 8], f32, tag="pooled2")
                nc.vector.max(out=t8a, in_=pooled)
                nc.vector.match_replace(out=pooled2, in_to_replace=t8a,
                                        in_values=pooled, imm_value=-3.0e38)
                nc.vector.max(out=t8b, in_=pooled2)
                idxq = tkpool.tile([C, 16], u16, tag="idxq")
                nc.vector.max_index(out=idxq[:, 0:8], in_max=t8a, in_values=S)
                nc.vector.max_index(out=idxq[:, 8:16], in_max=t8b, in_values=S)
                # widen to 8 replicas (u16->f32 casts), then one PE transpose
                # yields the "wrapped in 16 partitions, replicated per group"
                # layout directly (avoids SBUF->SBUF DMAs, which HW-deadlock
                # against in-flight xbar-transpose gathers)
                idxfR = tkpool.tile([C, 128], f32, tag="idxfR")
                for g in range(8):
                    nc.vector.tensor_copy(idxfR[:, g * 16:(g + 1) * 16], idxq)
                ptr = pps_t.tile([128, C], f32, tag="ptr")
                nc.tensor.transpose(ptr, idxfR, ident[:, :])
                nc.vector.tensor_copy(idxw[m][:, qt * 128:(qt + 1) * 128], ptr)
                # gather neighbor channel-rows from HBM (256B each), bf16,
                # landing channels-across-partitions: hG[:, 0, i] = h[idx_i, :]
                nc.gpsimd.dma_gather(
                    out_ap=hG[m][:, :, qt * 2048:(qt + 1) * 2048],
                    in_ap=xt_d[:, :],
                    idxs_ap=idxw[m][:, qt * 128:(qt + 1) * 128],
                    num_idxs=2048, num_idxs_reg=2048,
                    elem_size=C, transpose=True,
                    # >64 descriptors in one packet wedges the SDMA engine
                    single_packet=False)

        # iteration-1 MLP after all top-k emission: keeps the strict-FIFO
        # Vector queue free of reduces that would head-of-line-block the
        # next tile's top-k while waiting on gathers
        for qt in range(NQT):
            mlp_qt(qt, wzb, m_br)

        # ---------------- SE gate / iteration boundary ----------------
        gates = []
        m_br2 = [mpool.tile([C, QPC], f32, tag=f"m2_{b}", name=f"m2_{b}") for b in range(2)]
        for it in range(2):
            cur = m_br if it == 0 else m_br2
            packed = mpool.tile([C, 2], f32, tag=f"packed{it}")
            for b, bias in ((0, brgb), (1, bir)):
                mr = mpool.tile([C, QPC], f32, tag=f"mr{b}")
                nc.scalar.activation(mr, cur[b], AF.Relu, bias=bias,
                                     accum_out=packed[:, b:b + 1])
            # allreduce partial sums across the 4 cores of this sample
            red = mpool.tile([C, 2], f32, tag=f"red{it}")
            nc.sync.dma_start(out=cc_in[it][:, :], in_=packed)
            nc.gpsimd.collective_compute(
                "AllReduce", mybir.AluOpType.add, replica_groups=groups,
                ins=[cc_in[it][:, :]], outs=[cc_out[it][:, :]])
            nc.sync.dma_start(out=red, in_=cc_out[it][:, :])
            # SE gate
            p8 = pps_s.tile([C // 16, 1], f32, tag="p8")
            nc.tensor.matmul(p8, se1wa, red[:, 0:1], start=True, stop=False)
            nc.tensor.matmul(p8, se1wb, red[:, 1:2], start=False, stop=True)
            fc1 = mpool.tile([C // 16, 1], f32, tag="fc1")
            nc.scalar.activation(fc1, p8, AF.Relu, bias=se1b, scale=1.0 / HW)
            pg = pps_s.tile([C, 1], f32, tag="pg")
            nc.tensor.matmul(pg, se2w, fc1)
            gate = mpool.tile([C, 1], f32, tag=f"gate{it}")
            nc.scalar.activation(gate, pg, AF.Sigmoid, bias=se2b)
            gates.append(gate)
            if it == 0:
                # iteration 2: rescale weight rows by gate_1, rerun MLP
                wscaled = cpool.tile([C, 4 * C], bf16, tag="wscaled")
                nc.scalar.activation(wscaled, wz, AF.Copy, scale=gates[0])
                for qt in range(NQT):
                    mlp_qt(qt, wscaled, m_br2)

        # ---------------- final output ----------------
        gamb = mpool.tile([C, 1], f32, tag="gamb")
        nc.gpsimd.partition_broadcast(gamb, gam, channels=C)
        sfin = mpool.tile([C, 1], f32, tag="sfin")
        nc.vector.tensor_mul(sfin, gates[0], gates[1])
        nc.vector.tensor_mul(sfin, sfin, gamb)
        nc.vector.tensor_scalar_add(sfin, sfin, 1.0)
        outt = mpool.tile([C, QPC], f32, tag="outt")
        nc.scalar.activation(outt, Xq, AF.Relu, scale=sfin)
        nc.sync.dma_start(out=out_d[:, :], in_=outt)
        stack.close()

    nc.compile()
    return nc


def _get_nc():
    if "nc" not in _CACHE:
        _CACHE["nc"] = _build()
    return _CACHE["nc"]


def _shard_inputs(cnn_encoder_output, rgb, ir, rgb_g_w, rgb_g_b, ir_g_w, ir_g_b,
                  se1_w, se1_b, se2_w, se2_b, gamma):
    import ml_dtypes
    bf16 = ml_dtypes.bfloat16
    x = np.asarray(cnn_encoder_output, np.float32)
    rgbf = np.asarray(rgb, np.float32).reshape(N, 3, HW)
    irf = np.asarray(ir, np.float32).reshape(N, 1, HW)
    W1 = np.asarray(rgb_g_w, np.float32)[:C]
    W2 = np.asarray(rgb_g_w, np.float32)[C:]
    U1 = np.asarray(ir_g_w, np.float32)[:C]
    U2 = np.asarray(ir_g_w, np.float32)[C:]
    wz = np.concatenate([W1 + W2, -W2, U1 + U2, -U2], axis=1)  # [z1 | z2n | z3 | z4n]

    in_maps = []
    for core in range(NCORES):
        n, g = core // 4, core % 4
        sl = slice(g * QPC, (g + 1) * QPC)
        Xn = np.ascontiguousarray(x[n].reshape(C, HW))
        sq_rgb = (rgbf[n] ** 2).sum(axis=0)
        sq_ir = (irf[n] ** 2).sum(axis=0)
        # scores S' = 2<f_q, f_j> - sq_j  (descending S' == ascending distance)
        rgbq = np.concatenate([2.0 * rgbf[n][:, sl], -np.ones((1, QPC), np.float32)], 0)
        rgbk = np.concatenate([rgbf[n], sq_rgb[None]], 0)
        irq = np.concatenate([2.0 * irf[n][:, sl], -np.ones((1, QPC), np.float32)], 0)
        irk = np.concatenate([irf[n], sq_ir[None]], 0)
        in_maps.append({
            "xt": np.ascontiguousarray(Xn.T).astype(bf16),
            "xq": np.ascontiguousarray(Xn[:, sl]),
            "rgbq": np.ascontiguousarray(rgbq).astype(bf16),
            "rgbk": np.ascontiguousarray(rgbk).astype(bf16),
            "irq": np.ascontiguousarray(irq).astype(bf16),
            "irk": np.ascontiguousarray(irk).astype(bf16),
            "wz": np.ascontiguousarray(wz),
            "brgb": np.asarray(rgb_g_b, np.float32).reshape(C, 1),
            "bir": np.asarray(ir_g_b, np.float32).reshape(C, 1),
            "se1w": np.ascontiguousarray(np.asarray(se1_w, np.float32)),
            "se1b": np.asarray(se1_b, np.float32).reshape(C // 16, 1),
            "se2w": np.ascontiguousarray(np.asarray(se2_w, np.float32)),
            "se2b": np.asarray(se2_b, np.float32).reshape(C, 1),
            "gammav": np.asarray(gamma, np.float32).reshape(1, 1),
        })
    return in_maps


def kernel(cnn_encoder_output, rgb, ir, rgb_g_w, rgb_g_b, ir_g_w, ir_g_b,
           se1_w, se1_b, se2_w, se2_b, gamma, gnn_iterations, k,
           _debug=False, _trace=False):
    from concourse.bass_utils import run_bass_kernel_spmd

    assert int(gnn_iterations) == 2 and int(k) == 16
    nc = _get_nc()
    in_maps = _shard_inputs(cnn_encoder_output, rgb, ir, rgb_g_w, rgb_g_b,
                            ir_g_w, ir_g_b, se1_w, se1_b, se2_w, se2_b, gamma)
    kw = {}
    if _trace:
        kw = dict(trace=True)
    res = run_bass_kernel_spmd(nc, in_maps, core_ids=list(range(NCORES)), **kw)
    _CACHE["last_result"] = res
    out = np.empty((N, C, H, W), np.float32)
    for core in range(NCORES):
        n, g = core // 4, core % 4
        out[n].reshape(C, HW)[:, g * QPC:(g + 1) * QPC] = res.results[core]["out"]
    return out
